# revision 8
# baseline (speedup 1.0000x reference)
"""Multi-head scaled-cosine attention (B=2, L=2048, E=2048, H=16, D=128) on 8 trn2 cores.

Sharding: core c = (b, g) with b = batch (2), g = head-group of 4 heads (4 groups).
Each core computes its 4 heads' attention for its batch plus the partial output
projection; the host sums the 4 per-group partials per batch.

Precision: matmuls run in bf16 (PSUM accumulation is fp32). The Q/K RMS-norm
cancels exactly under the subsequent L2 normalization; the L2 reciprocal (and
logit scale) are folded into a diagonal matrix applied by the PE transpose that
produces Q^T/K^T. exp(bias - rowmax) is precomputed on the host and folded in
multiplicatively. Scores are built directly in [k, q] orientation so softmax
and attn@V need no on-chip transpose of the probability matrix. Q/K head dims
are host-permuted (evens|odds) so RoPE uses contiguous vector ops; the
permutation cancels inside q.k.

All bulk tensors are host-rearranged into their exact SBUF layouts so every DMA
moves multi-KB contiguous runs per partition (256B-line gathers made the
original startup DMA-bound). The softmax denominator is a bf16 running sum of
the per-kt probability tiles, split across the vector engine (odd kt) and the
otherwise-idle gpsimd engine (even kt), then reduced across partitions with one
ones-matmul per head. The per-head 1/den broadcast and the output projection of
each q-chunk are deferred into the next q-chunk's kt loop so the tensor engine
never waits on the Ln/Exp reciprocal chain.
"""
import sys
sys.path.insert(0, '/opt/trn_rl_repo')
import math
import numpy as np
import ml_dtypes

import concourse.bacc as bacc
import concourse.mybir as mybir
import concourse.tile as tile
from concourse.bass_utils import run_bass_kernel_spmd

F32 = mybir.dt.float32
F32R = mybir.dt.float32r
BF16 = mybir.dt.bfloat16
NP_BF16 = ml_dtypes.bfloat16
ALU = mybir.AluOpType
AF = mybir.ActivationFunctionType
AX = mybir.AxisListType

B, L, E, H, D = 2, 2048, 2048, 16, 128
G = 4                 # head groups
HPG = H // G          # heads per group = 4
GD = HPG * D          # 512, per-group projection width
P = 128               # partitions
NLT = L // P          # 16 l-tiles
NET = E // P          # 16 e-tiles (contraction)
NQC = L // 512        # 4 q-chunks
NKT = L // P          # 16 k-tiles
NB = 4                # x blocks of 4 l-tiles
DD = D // 2           # 64, per-head half dim
LOGIT_SCALE_MAX = math.log(1.0 / 0.01)


def _build(apply_qs: bool, apply_ks: bool):
    nc = bacc.Bacc(None, target_bir_lowering=False)
    d = {}
    # x / weights / sinusoids are pre-arranged on the host into SBUF layout
    d['xqP'] = nc.dram_tensor("xqP", [P, NB, NET, 512], BF16, kind="ExternalInput")
    d['xkvP'] = nc.dram_tensor("xkvP", [P, NB, NET, 512], BF16, kind="ExternalInput")
    d['expBT'] = nc.dram_tensor("expBT", [L, L], BF16, kind="ExternalInput")
    d['wqP'] = nc.dram_tensor("wqP", [P, NET, GD], BF16, kind="ExternalInput")
    d['wkP'] = nc.dram_tensor("wkP", [P, NET, GD], BF16, kind="ExternalInput")
    d['wvP'] = nc.dram_tensor("wvP", [P, NET, GD], BF16, kind="ExternalInput")
    d['woP'] = nc.dram_tensor("woP", [P, HPG, E], BF16, kind="ExternalInput")
    d['c4q'] = nc.dram_tensor("c4q", [P, NLT, DD], BF16, kind="ExternalInput")
    d['s4q'] = nc.dram_tensor("s4q", [P, NLT, DD], BF16, kind="ExternalInput")
    d['c4k'] = nc.dram_tensor("c4k", [P, NLT, DD], BF16, kind="ExternalInput")
    d['s4k'] = nc.dram_tensor("s4k", [P, NLT, DD], BF16, kind="ExternalInput")
    d['ls'] = nc.dram_tensor("ls", [P, HPG], F32, kind="ExternalInput")
    if apply_qs:
        d['qscale'] = nc.dram_tensor("qscale", [P, GD], F32, kind="ExternalInput")
    if apply_ks:
        d['kscale'] = nc.dram_tensor("kscale", [P, GD], F32, kind="ExternalInput")
    out = nc.dram_tensor("out", [L, E], BF16, kind="ExternalOutput")

    with tile.TileContext(nc) as tc:
        with tc.tile_pool(name="persist", bufs=1) as persist:
            qT = [persist.tile([P, L], BF16, tag=f"qT{h}", name=f"qT{h}") for h in range(HPG)]
            kT = [persist.tile([P, L], BF16, tag=f"kT{h}", name=f"kT{h}") for h in range(HPG)]
            v_sb = persist.tile([P, NLT, GD], BF16, tag="v_sb")

            w_all = {}
            for wname in ('wvP', 'wkP', 'wqP'):
                w_all[wname] = persist.tile([P, NET, GD], BF16, tag=wname, name=f"w_{wname}")
            # V weights first: the V pass can start after ~2.5MB of DMA
            nc.sync.dma_start(w_all['wvP'][:, 0:4, :], d['wvP'][:, 0:4, :])
            nc.sync.dma_start(w_all['wvP'][:, 4:NET, :], d['wvP'][:, 4:NET, :])
            ls_t = persist.tile([P, HPG], F32, tag="ls_t")
            nc.sync.dma_start(ls_t[:], d['ls'][:])

            identb = persist.tile([P, P], BF16, tag="identb")
            identf = persist.tile([P, P], F32, tag="identf")
            nc.vector.memset(identf[:], 0.0)
            nc.gpsimd.affine_select(out=identf[:], in_=identf[:],
                                    compare_op=ALU.not_equal, fill=1.0, base=0,
                                    pattern=[[-1, P]], channel_multiplier=1)
            nc.vector.tensor_copy(identb[:], identf[:])
            ones_f = persist.tile([P, P], F32, tag="ones_f")
            nc.vector.memset(ones_f[:], 1.0)
            ones_r = persist.tile([P, P], F32R, tag="ones_r")
            nc.scalar.copy(ones_r[:], ones_f[:])
            ones_b = persist.tile([P, P], BF16, tag="ones_b")
            nc.vector.tensor_copy(ones_b[:], ones_f[:])

            sin_sb = {}
            for sname in ('c4k', 's4k', 'c4q', 's4q'):
                sin_sb[sname] = persist.tile([P, NLT, DD], BF16, tag=sname,
                                             name=f"sin_{sname}")
            wo_sb = persist.tile([P, HPG, E], BF16, tag="wo_sb")

            qs_t = ks_t = None
            if apply_qs:
                qs_t = persist.tile([P, GD], F32, tag="qs_t")
                nc.sync.dma_start(qs_t[:], d['qscale'][:])
            if apply_ks:
                ks_t = persist.tile([P, GD], F32, tag="ks_t")
                nc.sync.dma_start(ks_t[:], d['kscale'][:])

            from contextlib import ExitStack
            proj_ctx = ExitStack()
            sbp = proj_ctx.enter_context(tc.tile_pool(name="proj_sb", bufs=2))
            nrm = proj_ctx.enter_context(tc.tile_pool(name="proj_nrm", bufs=4))
            psp = proj_ctx.enter_context(tc.tile_pool(name="proj_ps", bufs=3, space="PSUM"))
            pst = proj_ctx.enter_context(tc.tile_pool(name="proj_pst", bufs=3, space="PSUM"))

            # deferred loads dispatched from the Act engine during the V pass
            def _late_loads(step):
                if step == 0:
                    nc.scalar.dma_start(w_all['wkP'][:], d['wkP'][:])
                elif step == 1:
                    for sname in ('c4k', 's4k'):
                        nc.scalar.dma_start(sin_sb[sname][:], d[sname][:])
                elif step == 2:
                    nc.scalar.dma_start(w_all['wqP'][:], d['wqP'][:])
                elif step == 3:
                    for sname in ('c4q', 's4q'):
                        nc.scalar.dma_start(sin_sb[sname][:], d[sname][:])
                elif step == 4:
                    nc.scalar.dma_start(wo_sb[:], d['woP'][:])

            def load_block(x_dram, b, name):
                blk = sbp.tile([P, NET, 512], BF16, tag="xblk", name=name)
                nc.sync.dma_start(blk[:], x_dram[:, b, :, :])
                return blk

            def proj_mms(blk, s, w_sb, name):
                psum = psp.tile([P, GD], F32, tag="psum", name=name)
                for e in range(NET):
                    nc.tensor.matmul(psum[:], blk[:, e, s * P:(s + 1) * P],
                                     w_sb[:, e, :],
                                     start=(e == 0), stop=(e == NET - 1))
                return psum

            # ---------------- V pass ----------------
            for b in range(NB):
                blk = load_block(d['xkvP'], b, f"xvblk_{b}")
                if b == 0:
                    _late_loads(0)
                    _late_loads(1)
                elif b == 1:
                    _late_loads(2)
                    _late_loads(3)
                elif b == 2:
                    _late_loads(4)
                for s in range(4):
                    lt = 4 * b + s
                    psum_v = proj_mms(blk, s, w_all['wvP'], f"psumv_{lt}")
                    nc.scalar.copy(v_sb[:, lt, :], psum_v[:])

            # ---------------- Q/K norm helpers ----------------
            def norm_compute(lt, psum, cs, ss, scale_tile, use_ls, which):
                q1 = nrm.tile([P, GD], BF16, tag="q1")
                nc.scalar.copy(q1[:], psum[:])
                if scale_tile is not None:
                    nc.vector.tensor_mul(q1[:], q1[:], scale_tile[:])
                # per-head layout [evens(64) | odds(64)] (host-permuted weights)
                q1v = q1[:].rearrange("p (hh par dd) -> p hh par dd", hh=HPG, par=2)
                qe, qo = q1v[:, :, 0, :], q1v[:, :, 1, :]
                q2 = nrm.tile([P, GD], BF16, tag="q2")
                q2v = q2[:].rearrange("p (hh par dd) -> p hh par dd", hh=HPG, par=2)
                re, ro = q2v[:, :, 0, :], q2v[:, :, 1, :]
                ctv = cs[:, lt, :].rearrange("p (o d) -> p o d", o=1).broadcast_to([P, HPG, DD])
                stv = ss[:, lt, :].rearrange("p (o d) -> p o d", o=1).broadcast_to([P, HPG, DD])
                tmp = nrm.tile([P, HPG * DD], BF16, tag="tmp")
                tv = tmp[:].rearrange("p (hh dd) -> p hh dd", hh=HPG)
                # evens: qe*c - qo*s ; odds: qo*c + qe*s
                nc.vector.tensor_tensor(tv, qo, stv, ALU.mult)
                nc.vector.tensor_tensor(re, qe, ctv, ALU.mult)
                nc.vector.tensor_sub(re, re, tv)
                nc.vector.tensor_tensor(tv, qe, stv, ALU.mult)
                nc.vector.tensor_tensor(ro, qo, ctv, ALU.mult)
                nc.vector.tensor_add(ro, ro, tv)
                # L2 norm over each head's (now contiguous) D slice, on DVE
                sqs = nrm.tile([P, GD], BF16, tag="sqs")
                nc.vector.tensor_tensor(sqs[:], q2[:], q2[:], ALU.mult)
                acc = nrm.tile([P, HPG], F32, tag="acc")
                nc.vector.tensor_reduce(
                    acc[:], sqs[:].rearrange("p (hh dd) -> p hh dd", hh=HPG),
                    AX.X, ALU.add)
                nrm_t = nrm.tile([P, HPG], F32, tag="nrm_t")
                nc.scalar.activation(nrm_t[:], acc[:], AF.Sqrt)
                nc.vector.tensor_scalar_max(nrm_t[:], nrm_t[:], 1e-12)
                rcp = nrm.tile([P, HPG], F32, tag="rcp")
                nc.vector.reciprocal(rcp[:], nrm_t[:])
                if use_ls:
                    nc.vector.tensor_mul(rcp[:], rcp[:], ls_t[:])
                q3 = nrm.tile([P, GD], BF16, tag="q3", bufs=3, name=f"q3_{which}_{lt}")
                for h in range(HPG):
                    nc.vector.tensor_scalar_mul(q3[:, h * D:(h + 1) * D],
                                                q2[:, h * D:(h + 1) * D], rcp[:, h:h + 1])
                return q3

            def norm_transpose(lt, q3, dstT):
                for h in range(HPG):
                    pt = pst.tile([P, P], BF16, tag="pt", name=f"pt_{lt}_{h}")
                    nc.tensor.matmul(pt[:], q3[:, h * D:(h + 1) * D], identb[:],
                                     is_transpose=True)
                    nc.any.tensor_copy(dstT[h][:, lt * P:(lt + 1) * P], pt[:])

            # ---------------- K pass (transposes deferred one tile) ----------------
            prev = None
            for b in range(NB):
                blk = load_block(d['xkvP'], b, f"xkblk_{b}")
                for s in range(4):
                    lt = 4 * b + s
                    psum_k = proj_mms(blk, s, w_all['wkP'], f"psumk_{lt}")
                    q3 = norm_compute(lt, psum_k, sin_sb['c4k'], sin_sb['s4k'],
                                      ks_t, False, "k")
                    if prev is not None:
                        norm_transpose(prev[0], prev[1], kT)
                    prev = (lt, q3)
            norm_transpose(prev[0], prev[1], kT)

            # ---------------- Q pass ----------------
            prev = None
            for b in range(NB):
                blk = load_block(d['xqP'], b, f"xqblk_{b}")
                for s in range(4):
                    lt = 4 * b + s
                    psum_q = proj_mms(blk, s, w_all['wqP'], f"psumq_{lt}")
                    q3 = norm_compute(lt, psum_q, sin_sb['c4q'], sin_sb['s4q'],
                                      qs_t, True, "q")
                    if prev is not None:
                        norm_transpose(prev[0], prev[1], qT)
                    prev = (lt, q3)
            norm_transpose(prev[0], prev[1], qT)
            proj_ctx.close()

            # ---------------- attention per q-chunk ----------------
            att_ctx = ExitStack()
            asb = att_ctx.enter_context(tc.tile_pool(name="att_sb", bufs=3))
            aop = att_ctx.enter_context(tc.tile_pool(name="att_o", bufs=3))
            ps_pv = att_ctx.enter_context(tc.tile_pool(name="ps_pv", bufs=1, space="PSUM"))
            ps_s = att_ctx.enter_context(tc.tile_pool(name="ps_s", bufs=2, space="PSUM"))
            ps_o = att_ctx.enter_context(tc.tile_pool(name="ps_o", bufs=2, space="PSUM"))

            pending = []          # deferred outproj groups: (qc, pvc, lsub, ec)
            bps_todo = []         # deferred per-head 1/den broadcast+scale

            def outproj_group(qc, pvc, lsub, ec):
                o_ps = ps_o.tile([P, 512], F32, tag="ops", name=f"o{qc}_{lsub}_{ec}")
                for h in range(HPG):
                    nc.tensor.matmul(o_ps[:], pvc[h][:, lsub * P:(lsub + 1) * P],
                                     wo_sb[:, h, ec * 512:(ec + 1) * 512],
                                     start=(h == 0), stop=(h == HPG - 1))
                o_sb = aop.tile([P, 512], BF16, tag="o_sb", name=f"ob{qc}_{lsub}_{ec}")
                nc.vector.tensor_copy(o_sb[:], o_ps[:])
                nc.sync.dma_start(
                    out[qc * 512 + lsub * P: qc * 512 + (lsub + 1) * P,
                        ec * 512:(ec + 1) * 512], o_sb[:])

            def do_bps():
                qcb, pvc, rcp32s = bps_todo.pop(0)
                for h in range(HPG):
                    b_ps = ps_o.tile([P, 512], F32, tag="ops", name=f"b{qcb}_{h}")
                    off = 32 * (h % 2)
                    nc.tensor.matmul(b_ps[:], ones_r[off:off + 1, :],
                                     rcp32s[h // 2][off:off + 1, :],
                                     start=True, stop=True)
                    nc.vector.tensor_mul(pvc[h][:], pvc[h][:], b_ps[:])

            for qc in range(NQC):
                qsl = slice(qc * 512, (qc + 1) * 512)
                pv = [ps_pv.tile([P, 512], F32, tag=f"pv{h}", name=f"pv{qc}_{h}")
                      for h in range(HPG)]
                accD = asb.tile([P, 2048], BF16, tag="accD", bufs=2, name=f"accD{qc}")
                accP = asb.tile([P, 2048], BF16, tag="accP", bufs=2, name=f"accP{qc}")

                def stage1(kt, prev_pt):
                    eb = asb.tile([P, 512], BF16, tag="eb", name=f"eb{qc}_{kt}")
                    nc.sync.dma_start(eb[:], d['expBT'][kt * P:(kt + 1) * P, qsl])
                    p_t = asb.tile([P, 2048], BF16, tag="p_t", name=f"pt{qc}_{kt}", bufs=4)
                    s_ps = []
                    for h in range(HPG):
                        sp = ps_s.tile([P, 512], F32, tag="s", name=f"sp{qc}_{kt}_{h}")
                        s_ps.append(sp)
                        nc.tensor.matmul(sp[:], kT[h][:, kt * P:(kt + 1) * P],
                                         qT[h][:, qsl], start=True, stop=True)
                        nc.scalar.activation(p_t[:, h * 512:(h + 1) * 512], sp[:], AF.Exp)
                        if h == 1 and prev_pt is not None:
                            # slot 2-deep rotation: fill the PE with pv matmuls
                            # while the first two exps drain their PSUM slots
                            stage2(kt - 1, prev_pt)
                    ptv = p_t[:].rearrange("p (hh q) -> p hh q", hh=HPG)
                    ebb = eb[:].rearrange("p (o q) -> p o q", o=1).broadcast_to([P, HPG, 512])
                    nc.vector.tensor_tensor(ptv, ptv, ebb, ALU.mult)
                    # denominator running sum: odd kt on DVE, even kt on gpsimd
                    if kt == 0:
                        nc.gpsimd.tensor_copy(accP[:], p_t[:])
                    elif kt % 2 == 0 and kt <= 12:
                        nc.gpsimd.tensor_add(accP[:], accP[:], p_t[:])
                    elif kt == 1:
                        nc.vector.tensor_copy(accD[:], p_t[:])
                    else:
                        nc.vector.tensor_add(accD[:], accD[:], p_t[:])
                    return p_t

                def stage2(kt, p_t):
                    for h in range(HPG):
                        nc.tensor.matmul(pv[h][:], v_sb[:, kt, h * D:(h + 1) * D],
                                         p_t[:, h * 512:(h + 1) * 512],
                                         start=(kt == 0), stop=(kt == NKT - 1))

                prev_pt = None
                for kt in range(NKT):
                    p_t = stage1(kt, prev_pt)
                    if kt == 0 and bps_todo:
                        do_bps()
                    if kt >= 1 and pending:
                        outproj_group(*pending.pop(0))
                    prev_pt = p_t
                stage2(NKT - 1, prev_pt)
                while pending:
                    outproj_group(*pending.pop(0))

                # free pv banks early: copy unnormalized context to SBUF
                pvc = []
                for h in range(HPG):
                    c = asb.tile([P, 512], BF16, tag=f"pvc{h}", name=f"pvc{qc}_{h}", bufs=2)
                    nc.vector.tensor_copy(c[:], pv[h][:])
                    pvc.append(c)

                # combine the two denominator halves, then one cross-partition
                # ones-matmul per head; 1/den via exp(-ln(x))
                nc.vector.tensor_add(accD[:], accD[:], accP[:])
                den_ps = ps_o.tile([P, 512], F32, tag="ops", name=f"den{qc}")
                for h in range(HPG):
                    nc.tensor.matmul(den_ps[32 * h:32 * h + 32, :],
                                     ones_b[:, 0:32],
                                     accD[:, h * 512:(h + 1) * 512],
                                     start=True, stop=True,
                                     tile_position=(0, 32 * h))
                rcp32s = []
                for i in range(2):
                    lnd = asb.tile([64, 512], F32, tag=f"lnd{i}", bufs=2,
                                   name=f"lnd{qc}_{i}")
                    nc.scalar.activation(lnd[:], den_ps[64 * i:64 * i + 64, :], AF.Ln)
                    rcp32 = asb.tile([64, 512], F32R, tag=f"rcp32{i}", bufs=2,
                                     name=f"rcp32{qc}_{i}")
                    nc.scalar.activation(rcp32[:], lnd[:], AF.Exp, scale=-1.0)
                    rcp32s.append(rcp32)
                bps_todo.append((qc, pvc, rcp32s))
                for lsub in range(4):
                    for ec in range(4):
                        pending.append((qc, pvc, lsub, ec))
            do_bps()
            while pending:
                outproj_group(*pending.pop(0))
            att_ctx.close()
    nc.compile()
    return nc


# head-dim permutation: within each head, evens first then odds
_PERM = np.empty(GD, np.int64)
for _i in range(GD):
    _h, _j = divmod(_i, D)
    _par, _dd = divmod(_j, D // 2)
    _PERM[_i] = _h * D + 2 * _dd + _par


def _sbuf_layout(xT, inner):
    # [R, C] with R = n*128 -> [128, n, C] (partition-major, contiguous rows)
    n = xT.shape[0] // P
    return np.ascontiguousarray(xT.reshape(n, P, *inner).transpose(1, 0, *range(2, 2 + len(inner))))


def _prepare(inputs):
    f32 = np.float32
    inputs_q = np.asarray(inputs["inputs_q"], f32)
    inputs_kv = np.asarray(inputs["inputs_kv"], f32)
    bias = np.asarray(inputs["bias"], f32).reshape(L, L)
    q_sin = np.asarray(inputs["q_sinusoids"], f32)
    k_sin = np.asarray(inputs["k_sinusoids"], f32)
    Wq = np.asarray(inputs["Wq"], f32)
    Wk = np.asarray(inputs["Wk"], f32)
    Wv = np.asarray(inputs["Wv"], f32)
    Wo = np.asarray(inputs["Wo"], f32)
    qns = np.asarray(inputs["q_norm_scale"], f32)
    kns = np.asarray(inputs["k_norm_scale"], f32)
    ls = np.asarray(inputs["logit_scale"], f32)

    apply_qs = not np.all(qns == 1.0)
    apply_ks = not np.all(kns == 1.0)

    bm = bias.max(axis=1, keepdims=True)
    expBT = np.ascontiguousarray(np.exp((bias - bm).T).astype(NP_BF16))
    ls_e = np.exp(np.minimum(ls, LOGIT_SCALE_MAX)).astype(f32)

    def xprep(x):
        # [L, E] -> xT [E, L] -> [128, NB, NET, 512]: elem (p,b,g,l') =
        # xT[g*128+p, b*512+l']
        xT = x.T.astype(NP_BF16)
        return np.ascontiguousarray(
            xT.reshape(NET, P, NB, 512).transpose(1, 2, 0, 3))

    per_b = []
    for b in range(B):
        per_b.append(dict(
            xqP=xprep(inputs_q[b]),
            xkvP=xprep(inputs_kv[b]),
            c4q=_sbuf_layout(q_sin[b][:, 0::2].astype(NP_BF16), (DD,)),
            s4q=_sbuf_layout(q_sin[b][:, 1::2].astype(NP_BF16), (DD,)),
            c4k=_sbuf_layout(k_sin[b][:, 0::2].astype(NP_BF16), (DD,)),
            s4k=_sbuf_layout(k_sin[b][:, 1::2].astype(NP_BF16), (DD,)),
        ))
    per_g = []
    for g in range(G):
        rows = slice(g * GD, (g + 1) * GD)
        per_g.append(dict(
            wqP=_sbuf_layout(Wq[rows, :][_PERM, :].T.astype(NP_BF16), (GD,)),
            wkP=_sbuf_layout(Wk[rows, :][_PERM, :].T.astype(NP_BF16), (GD,)),
            wvP=_sbuf_layout(Wv[rows, :].T.astype(NP_BF16), (GD,)),
            woP=_sbuf_layout(Wo[:, rows].T.astype(NP_BF16), (E,)),
            ls=np.broadcast_to(ls_e[g * HPG:(g + 1) * HPG][None, :], (P, HPG)).copy(),
        ))

    qs_bc = (np.broadcast_to(np.tile(qns, HPG)[_PERM][None, :], (P, GD)).copy()
             if apply_qs else None)
    ks_bc = (np.broadcast_to(np.tile(kns, HPG)[_PERM][None, :], (P, GD)).copy()
             if apply_ks else None)

    in_maps = []
    for c in range(8):
        b, g = divmod(c, G)
        m = dict(expBT=expBT)
        m.update(per_b[b])
        m.update(per_g[g])
        if apply_qs:
            m['qscale'] = qs_bc
        if apply_ks:
            m['kscale'] = ks_bc
        in_maps.append(m)
    return in_maps, apply_qs, apply_ks


_CACHE = {}


def _get_nc(apply_qs, apply_ks):
    key = (apply_qs, apply_ks)
    if key not in _CACHE:
        _CACHE[key] = _build(apply_qs, apply_ks)
    return _CACHE[key]


def kernel(**inputs) -> np.ndarray:
    in_maps, apply_qs, apply_ks = _prepare(inputs)
    nc = _get_nc(apply_qs, apply_ks)
    res = run_bass_kernel_spmd(nc, in_maps, core_ids=list(range(8)))
    out = np.zeros((B, L, E), np.float32)
    for c in range(8):
        b = c // G
        out[b] += res.results[c]["out"].astype(np.float32)
    return out


# revision 15
# speedup vs baseline: 1.2161x; 1.2161x over previous
"""Multi-head scaled-cosine attention (B=2, L=2048, E=2048, H=16, D=128) on 8 trn2 cores.

Sharding: core c = (b, g) with b = batch (2), g = head-group of 4 heads (4 groups).
Each core computes its 4 heads' attention for its batch plus the partial output
projection; the host sums the 4 per-group partials per batch.

Precision: matmuls run in bf16 (PSUM accumulation is fp32). The Q/K RMS-norm
cancels exactly under the subsequent L2 normalization; the L2 reciprocal (and
logit scale) are folded into a diagonal matrix applied by the PE transpose that
produces Q^T/K^T. exp(bias - rowmax) is precomputed on the host and folded in
multiplicatively. Scores are built directly in [k, q] orientation so softmax
and attn@V need no on-chip transpose of the probability matrix. Q/K head dims
are host-permuted (evens|odds) so RoPE uses contiguous vector ops; the
permutation cancels inside q.k.

All bulk tensors are host-rearranged into their exact SBUF layouts so every DMA
moves multi-KB contiguous runs per partition; x is loaded per 128-row l-tile
(512KB, its own semaphore) so compute starts as soon as the first tile lands.
The softmax denominator folds the 16 probability tiles per q-chunk as bf16
pair-sums plus a running sum on the vector engine, and the tensor engine
reduces the few surviving tiles across partitions with accumulated
ones-matmuls. The per-head 1/den broadcast runs on the otherwise-idle gpsimd
engine (partition_broadcast); the output projection of each q-chunk is deferred
into the next q-chunk's kt loop so the tensor engine never waits on the Ln/Exp
reciprocal chain.
"""
import sys
sys.path.insert(0, '/opt/trn_rl_repo')
import math
import numpy as np
import ml_dtypes

import concourse.bacc as bacc
import concourse.mybir as mybir
import concourse.tile as tile
from concourse.bass_utils import run_bass_kernel_spmd

F32 = mybir.dt.float32
F32R = mybir.dt.float32r
BF16 = mybir.dt.bfloat16
NP_BF16 = ml_dtypes.bfloat16
ALU = mybir.AluOpType
AF = mybir.ActivationFunctionType
AX = mybir.AxisListType

B, L, E, H, D = 2, 2048, 2048, 16, 128
G = 4                 # head groups
HPG = H // G          # heads per group = 4
GD = HPG * D          # 512, per-group projection width
P = 128               # partitions
NLT = L // P          # 16 l-tiles
NET = E // P          # 16 e-tiles (contraction)
NQC = L // 512        # 4 q-chunks
NKT = L // P          # 16 k-tiles
NPAIR = 6             # kt pairs folded on DVE (kts 0..11); kts 12..15 direct
DD = D // 2           # 64, per-head half dim
LOGIT_SCALE_MAX = math.log(1.0 / 0.01)


def _build(apply_qs: bool, apply_ks: bool):
    nc = bacc.Bacc(None, target_bir_lowering=False)
    d = {}
    # x / weights / sinusoids are pre-arranged on the host into SBUF layout
    d['xqP'] = nc.dram_tensor("xqP", [P, NLT, NET, P], BF16, kind="ExternalInput")
    d['xkvP'] = nc.dram_tensor("xkvP", [P, NLT, NET, P], BF16, kind="ExternalInput")
    d['expBT'] = nc.dram_tensor("expBT", [L, L], BF16, kind="ExternalInput")
    d['wqP'] = nc.dram_tensor("wqP", [P, NET, GD], BF16, kind="ExternalInput")
    d['wkP'] = nc.dram_tensor("wkP", [P, NET, GD], BF16, kind="ExternalInput")
    d['wvP'] = nc.dram_tensor("wvP", [P, NET, GD], BF16, kind="ExternalInput")
    d['woP'] = nc.dram_tensor("woP", [P, HPG, E], BF16, kind="ExternalInput")
    d['c4q'] = nc.dram_tensor("c4q", [P, NLT, DD], BF16, kind="ExternalInput")
    d['s4q'] = nc.dram_tensor("s4q", [P, NLT, DD], BF16, kind="ExternalInput")
    d['c4k'] = nc.dram_tensor("c4k", [P, NLT, DD], BF16, kind="ExternalInput")
    d['s4k'] = nc.dram_tensor("s4k", [P, NLT, DD], BF16, kind="ExternalInput")
    d['ls'] = nc.dram_tensor("ls", [P, HPG], F32, kind="ExternalInput")
    if apply_qs:
        d['qscale'] = nc.dram_tensor("qscale", [P, GD], F32, kind="ExternalInput")
    if apply_ks:
        d['kscale'] = nc.dram_tensor("kscale", [P, GD], F32, kind="ExternalInput")
    out = nc.dram_tensor("out", [L, E], BF16, kind="ExternalOutput")

    with tile.TileContext(nc) as tc:
        with tc.tile_pool(name="persist", bufs=1) as persist:
            qT = [persist.tile([P, L], BF16, tag=f"qT{h}", name=f"qT{h}") for h in range(HPG)]
            kT = [persist.tile([P, L], BF16, tag=f"kT{h}", name=f"kT{h}") for h in range(HPG)]
            v_sb = persist.tile([P, NLT, GD], BF16, tag="v_sb")

            w_all = {}
            for wname in ('wvP', 'wkP', 'wqP'):
                w_all[wname] = persist.tile([P, NET, GD], BF16, tag=wname, name=f"w_{wname}")
            # V weights first: the V pass can start after ~1MB of DMA
            nc.sync.dma_start(w_all['wvP'][:, 0:4, :], d['wvP'][:, 0:4, :])
            nc.sync.dma_start(w_all['wvP'][:, 4:NET, :], d['wvP'][:, 4:NET, :])
            ls_t = persist.tile([P, HPG], F32, tag="ls_t")
            nc.sync.dma_start(ls_t[:], d['ls'][:])

            identb = persist.tile([P, P], BF16, tag="identb")
            identf = persist.tile([P, P], F32, tag="identf")
            nc.vector.memset(identf[:], 0.0)
            nc.gpsimd.affine_select(out=identf[:], in_=identf[:],
                                    compare_op=ALU.not_equal, fill=1.0, base=0,
                                    pattern=[[-1, P]], channel_multiplier=1)
            nc.vector.tensor_copy(identb[:], identf[:])
            ones_f = persist.tile([P, P], F32, tag="ones_f")
            nc.vector.memset(ones_f[:], 1.0)
            ones_b = persist.tile([P, P], BF16, tag="ones_b")
            nc.vector.tensor_copy(ones_b[:], ones_f[:])
            ones_r = persist.tile([P, P], F32R, tag="ones_r")
            nc.scalar.copy(ones_r[:], ones_f[:])

            sin_sb = {}
            for sname in ('c4k', 's4k', 'c4q', 's4q'):
                sin_sb[sname] = persist.tile([P, NLT, DD], BF16, tag=sname,
                                             name=f"sin_{sname}")
            wo_sb = persist.tile([P, HPG, E], BF16, tag="wo_sb")

            qs_t = ks_t = None
            if apply_qs:
                qs_t = persist.tile([P, GD], F32, tag="qs_t")
                nc.sync.dma_start(qs_t[:], d['qscale'][:])
            if apply_ks:
                ks_t = persist.tile([P, GD], F32, tag="ks_t")
                nc.sync.dma_start(ks_t[:], d['kscale'][:])

            from contextlib import ExitStack
            proj_ctx = ExitStack()
            sbp = proj_ctx.enter_context(tc.tile_pool(name="proj_sb", bufs=6))
            nrm = proj_ctx.enter_context(tc.tile_pool(name="proj_nrm", bufs=4))
            psp = proj_ctx.enter_context(tc.tile_pool(name="proj_ps", bufs=3, space="PSUM"))
            pst = proj_ctx.enter_context(tc.tile_pool(name="proj_pst", bufs=3, space="PSUM"))

            # deferred loads dispatched from the Act engine, spread out so they
            # never compete with the blocks the PE is about to need
            def _late_loads(step):
                if step == 0:
                    nc.scalar.dma_start(w_all['wkP'][:], d['wkP'][:])
                elif step == 1:
                    for sname in ('c4k', 's4k'):
                        nc.scalar.dma_start(sin_sb[sname][:], d[sname][:])
                elif step == 2:
                    nc.scalar.dma_start(w_all['wqP'][:], d['wqP'][:])
                elif step == 3:
                    for sname in ('c4q', 's4q'):
                        nc.scalar.dma_start(sin_sb[sname][:], d[sname][:])
                elif step == 4:
                    nc.scalar.dma_start(wo_sb[:], d['woP'][:])

            def load_tile(x_dram, lt, name):
                blk = sbp.tile([P, NET, P], BF16, tag="xblk", name=name)
                nc.sync.dma_start(blk[:], x_dram[:, lt, :, :])
                return blk

            def proj_mms(blk, w_sb, name):
                psum = psp.tile([P, GD], F32, tag="psum", name=name)
                for e in range(NET):
                    nc.tensor.matmul(psum[:], blk[:, e, :], w_sb[:, e, :],
                                     start=(e == 0), stop=(e == NET - 1))
                return psum

            # ---------------- V pass ----------------
            V_LATE = {4: 0, 8: 1, 10: 2, 12: 3, 14: 4}
            for lt in range(NLT):
                blk = load_tile(d['xkvP'], lt, f"xvblk_{lt}")
                if lt in V_LATE:
                    _late_loads(V_LATE[lt])
                psum_v = proj_mms(blk, w_all['wvP'], f"psumv_{lt}")
                nc.scalar.copy(v_sb[:, lt, :], psum_v[:])

            # ---------------- Q/K norm helpers ----------------
            def norm_compute(lt, psum, cs, ss, scale_tile, use_ls, which):
                q1 = nrm.tile([P, GD], BF16, tag="q1")
                nc.scalar.copy(q1[:], psum[:])
                if scale_tile is not None:
                    nc.vector.tensor_mul(q1[:], q1[:], scale_tile[:])
                # per-head layout [evens(64) | odds(64)] (host-permuted weights)
                q1v = q1[:].rearrange("p (hh par dd) -> p hh par dd", hh=HPG, par=2)
                qe, qo = q1v[:, :, 0, :], q1v[:, :, 1, :]
                q2 = nrm.tile([P, GD], BF16, tag="q2")
                q2v = q2[:].rearrange("p (hh par dd) -> p hh par dd", hh=HPG, par=2)
                re, ro = q2v[:, :, 0, :], q2v[:, :, 1, :]
                ctv = cs[:, lt, :].rearrange("p (o d) -> p o d", o=1).broadcast_to([P, HPG, DD])
                stv = ss[:, lt, :].rearrange("p (o d) -> p o d", o=1).broadcast_to([P, HPG, DD])
                tmp = nrm.tile([P, HPG * DD], BF16, tag="tmp")
                tv = tmp[:].rearrange("p (hh dd) -> p hh dd", hh=HPG)
                # evens: qe*c - qo*s ; odds: qo*c + qe*s
                nc.vector.tensor_tensor(tv, qo, stv, ALU.mult)
                nc.vector.tensor_tensor(re, qe, ctv, ALU.mult)
                nc.vector.tensor_sub(re, re, tv)
                nc.vector.tensor_tensor(tv, qe, stv, ALU.mult)
                nc.vector.tensor_tensor(ro, qo, ctv, ALU.mult)
                nc.vector.tensor_add(ro, ro, tv)
                # L2 norm over each head's (now contiguous) D slice, on DVE
                sqs = nrm.tile([P, GD], BF16, tag="sqs")
                nc.vector.tensor_tensor(sqs[:], q2[:], q2[:], ALU.mult)
                acc = nrm.tile([P, HPG], F32, tag="acc")
                nc.vector.tensor_reduce(
                    acc[:], sqs[:].rearrange("p (hh dd) -> p hh dd", hh=HPG),
                    AX.X, ALU.add)
                nrm_t = nrm.tile([P, HPG], F32, tag="nrm_t")
                nc.scalar.activation(nrm_t[:], acc[:], AF.Sqrt)
                nc.vector.tensor_scalar_max(nrm_t[:], nrm_t[:], 1e-12)
                rcp = nrm.tile([P, HPG], F32, tag="rcp")
                nc.vector.reciprocal(rcp[:], nrm_t[:])
                if use_ls:
                    nc.vector.tensor_mul(rcp[:], rcp[:], ls_t[:])
                q3 = nrm.tile([P, GD], BF16, tag="q3", bufs=3, name=f"q3_{which}_{lt}")
                for h in range(HPG):
                    nc.vector.tensor_scalar_mul(q3[:, h * D:(h + 1) * D],
                                                q2[:, h * D:(h + 1) * D], rcp[:, h:h + 1])
                return q3

            def norm_transpose(lt, q3, dstT):
                for h in range(HPG):
                    pt = pst.tile([P, P], BF16, tag="pt", name=f"pt_{lt}_{h}")
                    nc.tensor.matmul(pt[:], q3[:, h * D:(h + 1) * D], identb[:],
                                     is_transpose=True)
                    nc.any.tensor_copy(dstT[h][:, lt * P:(lt + 1) * P], pt[:])

            # ---------------- K pass (transposes deferred one tile) ----------------
            prev = None
            for lt in range(NLT):
                blk = load_tile(d['xkvP'], lt, f"xkblk_{lt}")
                psum_k = proj_mms(blk, w_all['wkP'], f"psumk_{lt}")
                q3 = norm_compute(lt, psum_k, sin_sb['c4k'], sin_sb['s4k'],
                                  ks_t, False, "k")
                if prev is not None:
                    norm_transpose(prev[0], prev[1], kT)
                prev = (lt, q3)
            norm_transpose(prev[0], prev[1], kT)

            # ---------------- Q pass ----------------
            prev = None
            for lt in range(NLT):
                blk = load_tile(d['xqP'], lt, f"xqblk_{lt}")
                psum_q = proj_mms(blk, w_all['wqP'], f"psumq_{lt}")
                q3 = norm_compute(lt, psum_q, sin_sb['c4q'], sin_sb['s4q'],
                                  qs_t, True, "q")
                if prev is not None:
                    norm_transpose(prev[0], prev[1], qT)
                prev = (lt, q3)
            norm_transpose(prev[0], prev[1], qT)
            proj_ctx.close()

            # ---------------- attention per q-chunk ----------------
            att_ctx = ExitStack()
            asb = att_ctx.enter_context(tc.tile_pool(name="att_sb", bufs=3))
            aop = att_ctx.enter_context(tc.tile_pool(name="att_o", bufs=3))
            ps_pv = att_ctx.enter_context(tc.tile_pool(name="ps_pv", bufs=1, space="PSUM"))
            ps_s = att_ctx.enter_context(tc.tile_pool(name="ps_s", bufs=2, space="PSUM"))
            ps_o = att_ctx.enter_context(tc.tile_pool(name="ps_o", bufs=2, space="PSUM"))

            pending = []          # deferred outproj groups: (qc, attn, lsub, ec)
            bps_todo = []         # deferred per-head 1/den broadcast+scale

            def outproj_group(qc, attn, lsub, ec):
                o_ps = ps_o.tile([P, 512], F32, tag="ops", name=f"o{qc}_{lsub}_{ec}")
                for h in range(HPG):
                    nc.tensor.matmul(o_ps[:], attn[h][:, lsub * P:(lsub + 1) * P],
                                     wo_sb[:, h, ec * 512:(ec + 1) * 512],
                                     start=(h == 0), stop=(h == HPG - 1))
                o_sb = aop.tile([P, 512], BF16, tag="o_sb", name=f"ob{qc}_{lsub}_{ec}")
                nc.vector.tensor_copy(o_sb[:], o_ps[:])
                nc.sync.dma_start(
                    out[qc * 512 + lsub * P: qc * 512 + (lsub + 1) * P,
                        ec * 512:(ec + 1) * 512], o_sb[:])

            def do_bps():
                # per-head 1/den broadcast over partitions by the PE, then
                # scale the copied context tiles
                qcb, pvc_b, attn_b, rcp32s = bps_todo.pop(0)
                for h in range(HPG):
                    b_ps = ps_o.tile([P, 512], F32, tag="ops", name=f"b{qcb}_{h}")
                    off = 32 * (h % 2)
                    nc.tensor.matmul(b_ps[:], ones_r[off:off + 1, :],
                                     rcp32s[h // 2][off:off + 1, :],
                                     start=True, stop=True)
                    nc.vector.tensor_tensor(attn_b[h][:], pvc_b[h][:], b_ps[:],
                                            ALU.mult)

            for qc in range(NQC):
                qsl = slice(qc * 512, (qc + 1) * 512)
                pv = [ps_pv.tile([P, 512], F32, tag=f"pv{h}", name=f"pv{qc}_{h}")
                      for h in range(HPG)]
                attn = [asb.tile([P, 512], BF16, tag=f"at{h}", bufs=2,
                                 name=f"at{qc}_{h}") for h in range(HPG)]
                acc = asb.tile([P, 2048], BF16, tag="den_acc", bufs=1,
                               name=f"dacc{qc}")
                prs = []

                def stage1(kt, prev_pt):
                    eb = asb.tile([P, 512], BF16, tag="eb", name=f"eb{qc}_{kt}")
                    nc.sync.dma_start(eb[:], d['expBT'][kt * P:(kt + 1) * P, qsl])
                    p_t = asb.tile([P, 2048], BF16, tag="p_t", name=f"pt{qc}_{kt}", bufs=5)
                    for h in range(HPG):
                        sp = ps_s.tile([P, 512], F32, tag="s", name=f"sp{qc}_{kt}_{h}")
                        nc.tensor.matmul(sp[:], kT[h][:, kt * P:(kt + 1) * P],
                                         qT[h][:, qsl], start=True, stop=True)
                        nc.scalar.activation(p_t[:, h * 512:(h + 1) * 512], sp[:], AF.Exp)
                        if h == 1 and prev_pt is not None:
                            # fill the PE with pv matmuls while the first two
                            # exps drain their PSUM slots
                            stage2(kt - 1, prev_pt)
                    ptv = p_t[:].rearrange("p (hh q) -> p hh q", hh=HPG)
                    ebb = eb[:].rearrange("p (o q) -> p o q", o=1).broadcast_to([P, HPG, 512])
                    nc.vector.tensor_tensor(ptv, ptv, ebb, ALU.mult)
                    # denominator folding on DVE: pair-sums for kts 0..11,
                    # then a running sum of the pairs; kts 12..15 go straight
                    # to the PE ones-matmuls
                    if kt < 2 * NPAIR and kt % 2 == 1:
                        pr = asb.tile([P, 2048], BF16, tag="pr", bufs=3,
                                      name=f"pr{qc}_{kt // 2}")
                        nc.vector.tensor_add(pr[:], prev_pt[:], p_t[:])
                        prs.append(pr)
                        if kt == 1:
                            pass
                        elif kt == 3:
                            nc.vector.tensor_add(acc[:], prs[0][:], prs[1][:])
                        else:
                            nc.vector.tensor_add(acc[:], acc[:], prs[kt // 2][:])
                    return p_t

                def stage2(kt, p_t):
                    for h in range(HPG):
                        nc.tensor.matmul(pv[h][:], v_sb[:, kt, h * D:(h + 1) * D],
                                         p_t[:, h * 512:(h + 1) * 512],
                                         start=(kt == 0), stop=(kt == NKT - 1))

                prev_pt = None
                tail_pts = []
                for kt in range(NKT):
                    p_t = stage1(kt, prev_pt)
                    if kt >= 2 * NPAIR:
                        tail_pts.append(p_t)
                    if kt == 0 and bps_todo:
                        do_bps()
                    if kt >= 1 and pending:
                        outproj_group(*pending.pop(0))
                    prev_pt = p_t
                stage2(NKT - 1, prev_pt)
                while pending:
                    outproj_group(*pending.pop(0))

                # free pv banks early: copy unnormalized context to SBUF
                pvc = []
                for h in range(HPG):
                    c = asb.tile([P, 512], BF16, tag=f"pvc{h}", name=f"pvc{qc}_{h}", bufs=2)
                    nc.vector.tensor_copy(c[:], pv[h][:])
                    pvc.append(c)

                # cross-partition denominator: accumulated ones-matmuls over
                # the folded acc plus the 4 unfolded tail tiles
                den_ps = ps_o.tile([P, 512], F32, tag="ops", name=f"den{qc}")
                den_rhs = [acc] + tail_pts
                for h in range(HPG):
                    for j, t in enumerate(den_rhs):
                        nc.tensor.matmul(den_ps[32 * h:32 * h + 32, :],
                                         ones_b[:, 0:32],
                                         t[:, h * 512:(h + 1) * 512],
                                         start=(j == 0), stop=(j == len(den_rhs) - 1),
                                         tile_position=(0, 32 * h))
                rcp32s = []
                for i in range(2):
                    lnd = asb.tile([64, 512], F32, tag=f"lnd{i}", bufs=2,
                                   name=f"lnd{qc}_{i}")
                    nc.scalar.activation(lnd[:], den_ps[64 * i:64 * i + 64, :], AF.Ln)
                    rcp32 = asb.tile([64, 512], F32R, tag=f"rcp32{i}", bufs=2,
                                     name=f"rcp32{qc}_{i}")
                    nc.scalar.activation(rcp32[:], lnd[:], AF.Exp, scale=-1.0)
                    rcp32s.append(rcp32)
                bps_todo.append((qc, pvc, attn, rcp32s))
                for lsub in range(4):
                    for ec in range(4):
                        pending.append((qc, attn, lsub, ec))
            do_bps()
            while pending:
                outproj_group(*pending.pop(0))
            att_ctx.close()
    nc.compile()
    return nc


# head-dim permutation: within each head, evens first then odds
_PERM = np.empty(GD, np.int64)
for _i in range(GD):
    _h, _j = divmod(_i, D)
    _par, _dd = divmod(_j, D // 2)
    _PERM[_i] = _h * D + 2 * _dd + _par


def _sbuf_layout(xT, inner):
    # [R, C] with R = n*128 -> [128, n, C] (partition-major, contiguous rows)
    n = xT.shape[0] // P
    return np.ascontiguousarray(xT.reshape(n, P, *inner).transpose(1, 0, *range(2, 2 + len(inner))))


def _prepare(inputs):
    f32 = np.float32
    inputs_q = np.asarray(inputs["inputs_q"], f32)
    inputs_kv = np.asarray(inputs["inputs_kv"], f32)
    bias = np.asarray(inputs["bias"], f32).reshape(L, L)
    q_sin = np.asarray(inputs["q_sinusoids"], f32)
    k_sin = np.asarray(inputs["k_sinusoids"], f32)
    Wq = np.asarray(inputs["Wq"], f32)
    Wk = np.asarray(inputs["Wk"], f32)
    Wv = np.asarray(inputs["Wv"], f32)
    Wo = np.asarray(inputs["Wo"], f32)
    qns = np.asarray(inputs["q_norm_scale"], f32)
    kns = np.asarray(inputs["k_norm_scale"], f32)
    ls = np.asarray(inputs["logit_scale"], f32)

    apply_qs = not np.all(qns == 1.0)
    apply_ks = not np.all(kns == 1.0)

    bm = bias.max(axis=1, keepdims=True)
    expBT = np.ascontiguousarray(np.exp((bias - bm).T).astype(NP_BF16))
    ls_e = np.exp(np.minimum(ls, LOGIT_SCALE_MAX)).astype(f32)

    def xprep(x):
        # [L, E] -> xT [E, L] -> [128, NLT, NET, 128]: elem (p,lt,g,l') =
        # xT[g*128+p, lt*128+l']
        xT = x.T.astype(NP_BF16)
        return np.ascontiguousarray(
            xT.reshape(NET, P, NLT, P).transpose(1, 2, 0, 3))

    per_b = []
    for b in range(B):
        per_b.append(dict(
            xqP=xprep(inputs_q[b]),
            xkvP=xprep(inputs_kv[b]),
            c4q=_sbuf_layout(q_sin[b][:, 0::2].astype(NP_BF16), (DD,)),
            s4q=_sbuf_layout(q_sin[b][:, 1::2].astype(NP_BF16), (DD,)),
            c4k=_sbuf_layout(k_sin[b][:, 0::2].astype(NP_BF16), (DD,)),
            s4k=_sbuf_layout(k_sin[b][:, 1::2].astype(NP_BF16), (DD,)),
        ))
    per_g = []
    for g in range(G):
        rows = slice(g * GD, (g + 1) * GD)
        per_g.append(dict(
            wqP=_sbuf_layout(Wq[rows, :][_PERM, :].T.astype(NP_BF16), (GD,)),
            wkP=_sbuf_layout(Wk[rows, :][_PERM, :].T.astype(NP_BF16), (GD,)),
            wvP=_sbuf_layout(Wv[rows, :].T.astype(NP_BF16), (GD,)),
            woP=_sbuf_layout(Wo[:, rows].T.astype(NP_BF16), (E,)),
            ls=np.broadcast_to(ls_e[g * HPG:(g + 1) * HPG][None, :], (P, HPG)).copy(),
        ))

    qs_bc = (np.broadcast_to(np.tile(qns, HPG)[_PERM][None, :], (P, GD)).copy()
             if apply_qs else None)
    ks_bc = (np.broadcast_to(np.tile(kns, HPG)[_PERM][None, :], (P, GD)).copy()
             if apply_ks else None)

    in_maps = []
    for c in range(8):
        b, g = divmod(c, G)
        m = dict(expBT=expBT)
        m.update(per_b[b])
        m.update(per_g[g])
        if apply_qs:
            m['qscale'] = qs_bc
        if apply_ks:
            m['kscale'] = ks_bc
        in_maps.append(m)
    return in_maps, apply_qs, apply_ks


_CACHE = {}


def _get_nc(apply_qs, apply_ks):
    key = (apply_qs, apply_ks)
    if key not in _CACHE:
        _CACHE[key] = _build(apply_qs, apply_ks)
    return _CACHE[key]


def kernel(**inputs) -> np.ndarray:
    in_maps, apply_qs, apply_ks = _prepare(inputs)
    nc = _get_nc(apply_qs, apply_ks)
    res = run_bass_kernel_spmd(nc, in_maps, core_ids=list(range(8)))
    out = np.zeros((B, L, E), np.float32)
    for c in range(8):
        b = c // G
        out[b] += res.results[c]["out"].astype(np.float32)
    return out


# revision 17
# speedup vs baseline: 1.2196x; 1.0028x over previous
"""Multi-head scaled-cosine attention (B=2, L=2048, E=2048, H=16, D=128) on 8 trn2 cores.

Sharding: core c = (b, g) with b = batch (2), g = head-group of 4 heads (4 groups).
Each core computes its 4 heads' attention for its batch plus the partial output
projection; the host sums the 4 per-group partials per batch.

Precision: matmuls run in bf16 (PSUM accumulation is fp32). The Q/K RMS-norm
cancels exactly under the subsequent L2 normalization; the L2 reciprocal (and
logit scale) are folded into a diagonal matrix applied by the PE transpose that
produces Q^T/K^T. exp(bias - rowmax) is precomputed on the host and folded in
multiplicatively. Scores are built directly in [k, q] orientation so softmax
and attn@V need no on-chip transpose of the probability matrix. Q/K head dims
are host-permuted (evens|odds) so RoPE uses contiguous vector ops; the
permutation cancels inside q.k.

All bulk tensors are host-rearranged into their exact SBUF layouts so every DMA
moves multi-KB contiguous runs per partition; x is loaded per 128-row l-tile
(512KB, its own semaphore) so compute starts as soon as the first tile lands.
The softmax denominator folds the 16 probability tiles per q-chunk as bf16
pair-sums plus a running sum on the vector engine, and the tensor engine
reduces the few surviving tiles across partitions with accumulated
ones-matmuls. The per-head 1/den broadcast runs on the otherwise-idle gpsimd
engine (partition_broadcast); the output projection of each q-chunk is deferred
into the next q-chunk's kt loop so the tensor engine never waits on the Ln/Exp
reciprocal chain.
"""
import sys
sys.path.insert(0, '/opt/trn_rl_repo')
import math
import numpy as np
import ml_dtypes

import concourse.bacc as bacc
import concourse.mybir as mybir
import concourse.tile as tile
from concourse.bass_utils import run_bass_kernel_spmd

F32 = mybir.dt.float32
F32R = mybir.dt.float32r
BF16 = mybir.dt.bfloat16
NP_BF16 = ml_dtypes.bfloat16
ALU = mybir.AluOpType
AF = mybir.ActivationFunctionType
AX = mybir.AxisListType

B, L, E, H, D = 2, 2048, 2048, 16, 128
G = 4                 # head groups
HPG = H // G          # heads per group = 4
GD = HPG * D          # 512, per-group projection width
P = 128               # partitions
NLT = L // P          # 16 l-tiles
NET = E // P          # 16 e-tiles (contraction)
NQC = L // 512        # 4 q-chunks
NKT = L // P          # 16 k-tiles
NPAIR = 6             # kt pairs folded on DVE (kts 0..11); kts 12..15 direct
DD = D // 2           # 64, per-head half dim
LOGIT_SCALE_MAX = math.log(1.0 / 0.01)


def _build(apply_qs: bool, apply_ks: bool):
    nc = bacc.Bacc(None, target_bir_lowering=False)
    d = {}
    # x / weights / sinusoids are pre-arranged on the host into SBUF layout
    d['xqP'] = nc.dram_tensor("xqP", [P, NLT, NET, P], BF16, kind="ExternalInput")
    d['xkvP'] = nc.dram_tensor("xkvP", [P, NLT, NET, P], BF16, kind="ExternalInput")
    d['expBT'] = nc.dram_tensor("expBT", [L, L], BF16, kind="ExternalInput")
    d['wqP'] = nc.dram_tensor("wqP", [P, NET, GD], BF16, kind="ExternalInput")
    d['wkP'] = nc.dram_tensor("wkP", [P, NET, GD], BF16, kind="ExternalInput")
    d['wvP'] = nc.dram_tensor("wvP", [P, NET, GD], BF16, kind="ExternalInput")
    d['woP'] = nc.dram_tensor("woP", [P, HPG, E], BF16, kind="ExternalInput")
    d['c4q'] = nc.dram_tensor("c4q", [P, NLT, DD], BF16, kind="ExternalInput")
    d['s4q'] = nc.dram_tensor("s4q", [P, NLT, DD], BF16, kind="ExternalInput")
    d['c4k'] = nc.dram_tensor("c4k", [P, NLT, DD], BF16, kind="ExternalInput")
    d['s4k'] = nc.dram_tensor("s4k", [P, NLT, DD], BF16, kind="ExternalInput")
    d['ls'] = nc.dram_tensor("ls", [P, HPG], F32, kind="ExternalInput")
    if apply_qs:
        d['qscale'] = nc.dram_tensor("qscale", [P, GD], F32, kind="ExternalInput")
    if apply_ks:
        d['kscale'] = nc.dram_tensor("kscale", [P, GD], F32, kind="ExternalInput")
    out = nc.dram_tensor("out", [L, E], BF16, kind="ExternalOutput")

    with tile.TileContext(nc) as tc:
        with tc.tile_pool(name="persist", bufs=1) as persist:
            qT = [persist.tile([P, L], BF16, tag=f"qT{h}", name=f"qT{h}") for h in range(HPG)]
            kT = [persist.tile([P, L], BF16, tag=f"kT{h}", name=f"kT{h}") for h in range(HPG)]
            v_sb = persist.tile([P, NLT, GD], BF16, tag="v_sb")

            w_all = {}
            for wname in ('wvP', 'wkP', 'wqP'):
                w_all[wname] = persist.tile([P, NET, GD], BF16, tag=wname, name=f"w_{wname}")
            # V weights first: the V pass can start after ~1MB of DMA.
            # Only the first chunk rides the sync queue ahead of the x tiles;
            # the rest goes via the Act queue so it can't delay tile 0.
            nc.sync.dma_start(w_all['wvP'][:, 0:4, :], d['wvP'][:, 0:4, :])
            nc.scalar.dma_start(w_all['wvP'][:, 4:NET, :], d['wvP'][:, 4:NET, :])
            ls_t = persist.tile([P, HPG], F32, tag="ls_t")
            nc.scalar.dma_start(ls_t[:], d['ls'][:])

            identb = persist.tile([P, P], BF16, tag="identb")
            identf = persist.tile([P, P], F32, tag="identf")
            nc.vector.memset(identf[:], 0.0)
            nc.gpsimd.affine_select(out=identf[:], in_=identf[:],
                                    compare_op=ALU.not_equal, fill=1.0, base=0,
                                    pattern=[[-1, P]], channel_multiplier=1)
            nc.vector.tensor_copy(identb[:], identf[:])
            ones_f = persist.tile([P, P], F32, tag="ones_f")
            nc.vector.memset(ones_f[:], 1.0)
            ones_b = persist.tile([P, P], BF16, tag="ones_b")
            nc.vector.tensor_copy(ones_b[:], ones_f[:])
            ones_r = persist.tile([P, P], F32R, tag="ones_r")
            nc.scalar.copy(ones_r[:], ones_f[:])

            sin_sb = {}
            for sname in ('c4k', 's4k', 'c4q', 's4q'):
                sin_sb[sname] = persist.tile([P, NLT, DD], BF16, tag=sname,
                                             name=f"sin_{sname}")
            wo_sb = persist.tile([P, HPG, E], BF16, tag="wo_sb")

            qs_t = ks_t = None
            if apply_qs:
                qs_t = persist.tile([P, GD], F32, tag="qs_t")
                nc.sync.dma_start(qs_t[:], d['qscale'][:])
            if apply_ks:
                ks_t = persist.tile([P, GD], F32, tag="ks_t")
                nc.sync.dma_start(ks_t[:], d['kscale'][:])

            from contextlib import ExitStack
            proj_ctx = ExitStack()
            sbp = proj_ctx.enter_context(tc.tile_pool(name="proj_sb", bufs=6))
            nrm = proj_ctx.enter_context(tc.tile_pool(name="proj_nrm", bufs=4))
            psp = proj_ctx.enter_context(tc.tile_pool(name="proj_ps", bufs=3, space="PSUM"))
            pst = proj_ctx.enter_context(tc.tile_pool(name="proj_pst", bufs=3, space="PSUM"))

            # deferred loads dispatched from the Act engine, spread out so they
            # never compete with the blocks the PE is about to need
            def _late_loads(step):
                if step == 0:
                    nc.scalar.dma_start(w_all['wkP'][:], d['wkP'][:])
                elif step == 1:
                    for sname in ('c4k', 's4k'):
                        nc.scalar.dma_start(sin_sb[sname][:], d[sname][:])
                elif step == 2:
                    nc.scalar.dma_start(w_all['wqP'][:], d['wqP'][:])
                elif step == 3:
                    for sname in ('c4q', 's4q'):
                        nc.scalar.dma_start(sin_sb[sname][:], d[sname][:])
                elif step == 4:
                    nc.scalar.dma_start(wo_sb[:], d['woP'][:])

            def load_tile(x_dram, lt, name):
                blk = sbp.tile([P, NET, P], BF16, tag="xblk", name=name)
                nc.sync.dma_start(blk[:], x_dram[:, lt, :, :])
                return blk

            def proj_mms(blk, w_sb, name):
                psum = psp.tile([P, GD], F32, tag="psum", name=name)
                for e in range(NET):
                    nc.tensor.matmul(psum[:], blk[:, e, :], w_sb[:, e, :],
                                     start=(e == 0), stop=(e == NET - 1))
                return psum

            # ---------------- V pass ----------------
            V_LATE = {4: 0, 8: 1, 10: 2, 12: 3, 14: 4}
            for lt in range(NLT):
                blk = load_tile(d['xkvP'], lt, f"xvblk_{lt}")
                if lt in V_LATE:
                    _late_loads(V_LATE[lt])
                psum_v = proj_mms(blk, w_all['wvP'], f"psumv_{lt}")
                nc.scalar.copy(v_sb[:, lt, :], psum_v[:])

            # ---------------- Q/K norm helpers ----------------
            def norm_compute(lt, psum, cs, ss, scale_tile, use_ls, which):
                q1 = nrm.tile([P, GD], BF16, tag="q1")
                nc.scalar.copy(q1[:], psum[:])
                if scale_tile is not None:
                    nc.vector.tensor_mul(q1[:], q1[:], scale_tile[:])
                # per-head layout [evens(64) | odds(64)] (host-permuted weights)
                q1v = q1[:].rearrange("p (hh par dd) -> p hh par dd", hh=HPG, par=2)
                qe, qo = q1v[:, :, 0, :], q1v[:, :, 1, :]
                q2 = nrm.tile([P, GD], BF16, tag="q2")
                q2v = q2[:].rearrange("p (hh par dd) -> p hh par dd", hh=HPG, par=2)
                re, ro = q2v[:, :, 0, :], q2v[:, :, 1, :]
                ctv = cs[:, lt, :].rearrange("p (o d) -> p o d", o=1).broadcast_to([P, HPG, DD])
                stv = ss[:, lt, :].rearrange("p (o d) -> p o d", o=1).broadcast_to([P, HPG, DD])
                tmp = nrm.tile([P, HPG * DD], BF16, tag="tmp")
                tv = tmp[:].rearrange("p (hh dd) -> p hh dd", hh=HPG)
                # evens: qe*c - qo*s ; odds: qo*c + qe*s
                nc.vector.tensor_tensor(tv, qo, stv, ALU.mult)
                nc.vector.tensor_tensor(re, qe, ctv, ALU.mult)
                nc.vector.tensor_sub(re, re, tv)
                nc.vector.tensor_tensor(tv, qe, stv, ALU.mult)
                nc.vector.tensor_tensor(ro, qo, ctv, ALU.mult)
                nc.vector.tensor_add(ro, ro, tv)
                # L2 norm over each head's (now contiguous) D slice, on DVE
                sqs = nrm.tile([P, GD], BF16, tag="sqs")
                nc.vector.tensor_tensor(sqs[:], q2[:], q2[:], ALU.mult)
                acc = nrm.tile([P, HPG], F32, tag="acc")
                nc.vector.tensor_reduce(
                    acc[:], sqs[:].rearrange("p (hh dd) -> p hh dd", hh=HPG),
                    AX.X, ALU.add)
                nrm_t = nrm.tile([P, HPG], F32, tag="nrm_t")
                nc.scalar.activation(nrm_t[:], acc[:], AF.Sqrt)
                nc.vector.tensor_scalar_max(nrm_t[:], nrm_t[:], 1e-12)
                rcp = nrm.tile([P, HPG], F32, tag="rcp")
                nc.vector.reciprocal(rcp[:], nrm_t[:])
                if use_ls:
                    nc.vector.tensor_mul(rcp[:], rcp[:], ls_t[:])
                q3 = nrm.tile([P, GD], BF16, tag="q3", bufs=3, name=f"q3_{which}_{lt}")
                for h in range(HPG):
                    nc.vector.tensor_scalar_mul(q3[:, h * D:(h + 1) * D],
                                                q2[:, h * D:(h + 1) * D], rcp[:, h:h + 1])
                return q3

            def norm_transpose(lt, q3, dstT):
                for h in range(HPG):
                    pt = pst.tile([P, P], BF16, tag="pt", name=f"pt_{lt}_{h}")
                    nc.tensor.matmul(pt[:], q3[:, h * D:(h + 1) * D], identb[:],
                                     is_transpose=True)
                    nc.any.tensor_copy(dstT[h][:, lt * P:(lt + 1) * P], pt[:])

            # ---------------- K pass (transposes deferred one tile) ----------------
            prev = None
            for lt in range(NLT):
                blk = load_tile(d['xkvP'], lt, f"xkblk_{lt}")
                psum_k = proj_mms(blk, w_all['wkP'], f"psumk_{lt}")
                q3 = norm_compute(lt, psum_k, sin_sb['c4k'], sin_sb['s4k'],
                                  ks_t, False, "k")
                if prev is not None:
                    norm_transpose(prev[0], prev[1], kT)
                prev = (lt, q3)
            norm_transpose(prev[0], prev[1], kT)

            # ---------------- Q pass ----------------
            prev = None
            for lt in range(NLT):
                blk = load_tile(d['xqP'], lt, f"xqblk_{lt}")
                psum_q = proj_mms(blk, w_all['wqP'], f"psumq_{lt}")
                q3 = norm_compute(lt, psum_q, sin_sb['c4q'], sin_sb['s4q'],
                                  qs_t, True, "q")
                if prev is not None:
                    norm_transpose(prev[0], prev[1], qT)
                prev = (lt, q3)
            norm_transpose(prev[0], prev[1], qT)
            proj_ctx.close()

            # ---------------- attention per q-chunk ----------------
            att_ctx = ExitStack()
            asb = att_ctx.enter_context(tc.tile_pool(name="att_sb", bufs=3))
            aop = att_ctx.enter_context(tc.tile_pool(name="att_o", bufs=3))
            ps_pv = att_ctx.enter_context(tc.tile_pool(name="ps_pv", bufs=1, space="PSUM"))
            ps_s = att_ctx.enter_context(tc.tile_pool(name="ps_s", bufs=2, space="PSUM"))
            ps_o = att_ctx.enter_context(tc.tile_pool(name="ps_o", bufs=2, space="PSUM"))

            pending = []          # deferred outproj groups: (qc, attn, lsub, ec)
            bps_todo = []         # deferred per-head 1/den broadcast+scale

            def outproj_group(qc, attn, lsub, ec):
                o_ps = ps_o.tile([P, 512], F32, tag="ops", name=f"o{qc}_{lsub}_{ec}")
                for h in range(HPG):
                    nc.tensor.matmul(o_ps[:], attn[h][:, lsub * P:(lsub + 1) * P],
                                     wo_sb[:, h, ec * 512:(ec + 1) * 512],
                                     start=(h == 0), stop=(h == HPG - 1))
                o_sb = aop.tile([P, 512], BF16, tag="o_sb", name=f"ob{qc}_{lsub}_{ec}")
                # alternate the PSUM drain between DVE and Act: the vector
                # engine is oversubscribed on the odd (pair-folding) kts
                if (lsub * 4 + ec) % 2 == 0:
                    nc.vector.tensor_copy(o_sb[:], o_ps[:])
                else:
                    nc.scalar.copy(o_sb[:], o_ps[:])
                nc.sync.dma_start(
                    out[qc * 512 + lsub * P: qc * 512 + (lsub + 1) * P,
                        ec * 512:(ec + 1) * 512], o_sb[:])

            def do_bps():
                # per-head 1/den broadcast over partitions by the PE, then
                # scale the copied context tiles
                qcb, pvc_b, attn_b, rcp32s = bps_todo.pop(0)
                for h in range(HPG):
                    b_ps = ps_o.tile([P, 512], F32, tag="ops", name=f"b{qcb}_{h}")
                    off = 32 * (h % 2)
                    nc.tensor.matmul(b_ps[:], ones_r[off:off + 1, :],
                                     rcp32s[h // 2][off:off + 1, :],
                                     start=True, stop=True)
                    nc.vector.tensor_tensor(attn_b[h][:], pvc_b[h][:], b_ps[:],
                                            ALU.mult)

            for qc in range(NQC):
                qsl = slice(qc * 512, (qc + 1) * 512)
                pv = [ps_pv.tile([P, 512], F32, tag=f"pv{h}", name=f"pv{qc}_{h}")
                      for h in range(HPG)]
                attn = [asb.tile([P, 512], BF16, tag=f"at{h}", bufs=2,
                                 name=f"at{qc}_{h}") for h in range(HPG)]
                acc = asb.tile([P, 2048], BF16, tag="den_acc", bufs=1,
                               name=f"dacc{qc}")
                prs = []

                def stage1(kt, prev_pt):
                    eb = asb.tile([P, 512], BF16, tag="eb", name=f"eb{qc}_{kt}")
                    nc.sync.dma_start(eb[:], d['expBT'][kt * P:(kt + 1) * P, qsl])
                    p_t = asb.tile([P, 2048], BF16, tag="p_t", name=f"pt{qc}_{kt}", bufs=5)
                    for h in range(HPG):
                        sp = ps_s.tile([P, 512], F32, tag="s", name=f"sp{qc}_{kt}_{h}")
                        nc.tensor.matmul(sp[:], kT[h][:, kt * P:(kt + 1) * P],
                                         qT[h][:, qsl], start=True, stop=True)
                        nc.scalar.activation(p_t[:, h * 512:(h + 1) * 512], sp[:], AF.Exp)
                        if h == 1 and prev_pt is not None:
                            # fill the PE with pv matmuls while the first two
                            # exps drain their PSUM slots
                            stage2(kt - 1, prev_pt)
                    ptv = p_t[:].rearrange("p (hh q) -> p hh q", hh=HPG)
                    ebb = eb[:].rearrange("p (o q) -> p o q", o=1).broadcast_to([P, HPG, 512])
                    nc.vector.tensor_tensor(ptv, ptv, ebb, ALU.mult)
                    # denominator folding on DVE: pair-sums for kts 0..11,
                    # then a running sum of the pairs; kts 12..15 go straight
                    # to the PE ones-matmuls
                    if kt < 2 * NPAIR and kt % 2 == 1:
                        pr = asb.tile([P, 2048], BF16, tag="pr", bufs=3,
                                      name=f"pr{qc}_{kt // 2}")
                        nc.vector.tensor_add(pr[:], prev_pt[:], p_t[:])
                        prs.append(pr)
                        if kt == 1:
                            pass
                        elif kt == 3:
                            nc.vector.tensor_add(acc[:], prs[0][:], prs[1][:])
                        else:
                            nc.vector.tensor_add(acc[:], acc[:], prs[kt // 2][:])
                    return p_t

                def stage2(kt, p_t):
                    for h in range(HPG):
                        nc.tensor.matmul(pv[h][:], v_sb[:, kt, h * D:(h + 1) * D],
                                         p_t[:, h * 512:(h + 1) * 512],
                                         start=(kt == 0), stop=(kt == NKT - 1))

                prev_pt = None
                tail_pts = []
                for kt in range(NKT):
                    p_t = stage1(kt, prev_pt)
                    if kt >= 2 * NPAIR:
                        tail_pts.append(p_t)
                    if kt == 0 and bps_todo:
                        do_bps()
                    if kt >= 1 and pending:
                        outproj_group(*pending.pop(0))
                    prev_pt = p_t
                stage2(NKT - 1, prev_pt)
                while pending:
                    outproj_group(*pending.pop(0))

                # free pv banks early: copy unnormalized context to SBUF
                pvc = []
                for h in range(HPG):
                    c = asb.tile([P, 512], BF16, tag=f"pvc{h}", name=f"pvc{qc}_{h}", bufs=2)
                    nc.vector.tensor_copy(c[:], pv[h][:])
                    pvc.append(c)

                # cross-partition denominator: accumulated ones-matmuls over
                # the folded acc plus the 4 unfolded tail tiles
                den_ps = ps_o.tile([P, 512], F32, tag="ops", name=f"den{qc}")
                den_rhs = [acc] + tail_pts
                for h in range(HPG):
                    for j, t in enumerate(den_rhs):
                        nc.tensor.matmul(den_ps[32 * h:32 * h + 32, :],
                                         ones_b[:, 0:32],
                                         t[:, h * 512:(h + 1) * 512],
                                         start=(j == 0), stop=(j == len(den_rhs) - 1),
                                         tile_position=(0, 32 * h))
                rcp32s = []
                for i in range(2):
                    lnd = asb.tile([64, 512], F32, tag=f"lnd{i}", bufs=2,
                                   name=f"lnd{qc}_{i}")
                    nc.scalar.activation(lnd[:], den_ps[64 * i:64 * i + 64, :], AF.Ln)
                    rcp32 = asb.tile([64, 512], F32R, tag=f"rcp32{i}", bufs=2,
                                     name=f"rcp32{qc}_{i}")
                    nc.scalar.activation(rcp32[:], lnd[:], AF.Exp, scale=-1.0)
                    rcp32s.append(rcp32)
                bps_todo.append((qc, pvc, attn, rcp32s))
                for lsub in range(4):
                    for ec in range(4):
                        pending.append((qc, attn, lsub, ec))
            do_bps()
            while pending:
                outproj_group(*pending.pop(0))
            att_ctx.close()
    nc.compile()
    return nc


# head-dim permutation: within each head, evens first then odds
_PERM = np.empty(GD, np.int64)
for _i in range(GD):
    _h, _j = divmod(_i, D)
    _par, _dd = divmod(_j, D // 2)
    _PERM[_i] = _h * D + 2 * _dd + _par


def _sbuf_layout(xT, inner):
    # [R, C] with R = n*128 -> [128, n, C] (partition-major, contiguous rows)
    n = xT.shape[0] // P
    return np.ascontiguousarray(xT.reshape(n, P, *inner).transpose(1, 0, *range(2, 2 + len(inner))))


def _prepare(inputs):
    f32 = np.float32
    inputs_q = np.asarray(inputs["inputs_q"], f32)
    inputs_kv = np.asarray(inputs["inputs_kv"], f32)
    bias = np.asarray(inputs["bias"], f32).reshape(L, L)
    q_sin = np.asarray(inputs["q_sinusoids"], f32)
    k_sin = np.asarray(inputs["k_sinusoids"], f32)
    Wq = np.asarray(inputs["Wq"], f32)
    Wk = np.asarray(inputs["Wk"], f32)
    Wv = np.asarray(inputs["Wv"], f32)
    Wo = np.asarray(inputs["Wo"], f32)
    qns = np.asarray(inputs["q_norm_scale"], f32)
    kns = np.asarray(inputs["k_norm_scale"], f32)
    ls = np.asarray(inputs["logit_scale"], f32)

    apply_qs = not np.all(qns == 1.0)
    apply_ks = not np.all(kns == 1.0)

    bm = bias.max(axis=1, keepdims=True)
    expBT = np.ascontiguousarray(np.exp((bias - bm).T).astype(NP_BF16))
    ls_e = np.exp(np.minimum(ls, LOGIT_SCALE_MAX)).astype(f32)

    def xprep(x):
        # [L, E] -> xT [E, L] -> [128, NLT, NET, 128]: elem (p,lt,g,l') =
        # xT[g*128+p, lt*128+l']
        xT = x.T.astype(NP_BF16)
        return np.ascontiguousarray(
            xT.reshape(NET, P, NLT, P).transpose(1, 2, 0, 3))

    per_b = []
    for b in range(B):
        per_b.append(dict(
            xqP=xprep(inputs_q[b]),
            xkvP=xprep(inputs_kv[b]),
            c4q=_sbuf_layout(q_sin[b][:, 0::2].astype(NP_BF16), (DD,)),
            s4q=_sbuf_layout(q_sin[b][:, 1::2].astype(NP_BF16), (DD,)),
            c4k=_sbuf_layout(k_sin[b][:, 0::2].astype(NP_BF16), (DD,)),
            s4k=_sbuf_layout(k_sin[b][:, 1::2].astype(NP_BF16), (DD,)),
        ))
    per_g = []
    for g in range(G):
        rows = slice(g * GD, (g + 1) * GD)
        per_g.append(dict(
            wqP=_sbuf_layout(Wq[rows, :][_PERM, :].T.astype(NP_BF16), (GD,)),
            wkP=_sbuf_layout(Wk[rows, :][_PERM, :].T.astype(NP_BF16), (GD,)),
            wvP=_sbuf_layout(Wv[rows, :].T.astype(NP_BF16), (GD,)),
            woP=_sbuf_layout(Wo[:, rows].T.astype(NP_BF16), (E,)),
            ls=np.broadcast_to(ls_e[g * HPG:(g + 1) * HPG][None, :], (P, HPG)).copy(),
        ))

    qs_bc = (np.broadcast_to(np.tile(qns, HPG)[_PERM][None, :], (P, GD)).copy()
             if apply_qs else None)
    ks_bc = (np.broadcast_to(np.tile(kns, HPG)[_PERM][None, :], (P, GD)).copy()
             if apply_ks else None)

    in_maps = []
    for c in range(8):
        b, g = divmod(c, G)
        m = dict(expBT=expBT)
        m.update(per_b[b])
        m.update(per_g[g])
        if apply_qs:
            m['qscale'] = qs_bc
        if apply_ks:
            m['kscale'] = ks_bc
        in_maps.append(m)
    return in_maps, apply_qs, apply_ks


_CACHE = {}


def _get_nc(apply_qs, apply_ks):
    key = (apply_qs, apply_ks)
    if key not in _CACHE:
        _CACHE[key] = _build(apply_qs, apply_ks)
    return _CACHE[key]


def kernel(**inputs) -> np.ndarray:
    in_maps, apply_qs, apply_ks = _prepare(inputs)
    nc = _get_nc(apply_qs, apply_ks)
    res = run_bass_kernel_spmd(nc, in_maps, core_ids=list(range(8)))
    out = np.zeros((B, L, E), np.float32)
    for c in range(8):
        b = c // G
        out[b] += res.results[c]["out"].astype(np.float32)
    return out
